# revision 1
# baseline (speedup 1.0000x reference)
"""DGCNN segmentation (3x EdgeConv max-aggregation + MLP head) on 8 Trainium2 cores.

Sharding: nodes are split into 8 equal contiguous blocks (one per core); each
core owns all edges whose *destination* lies in its block, so the scatter-max
aggregation is core-local.  Per-layer node tables (the per-node halves of the
first edge-MLP linear) are computed data-parallel over nodes and AllGather'd so
every core can gather any source node's contribution.

Per layer l (C_in -> C -> C, PyG EdgeConv):
    m_e   = relu(u[dst_e] + v[src_e] + ba_l) @ Wb_l          (per edge)
    h_i   = max_{e: dst_e = i} m_e + bb_l   (0 if no edges)
  where u = h @ (Wa_l[:C_in] - Wa_l[C_in:]),  v = h @ Wa_l[C_in:].

Device pipeline per core:
  - u/v row-tables in HBM (bf16); per-edge transposed gather via
    gpsimd.dma_gather(transpose=True) puts channels on partitions.
  - scalar_tensor_tensor adds (u + ba) + v, scalar-engine relu, PE matmul with
    Wb, then a segmented max over each node's padded slot-block via pool_max.
  - Edges are pre-sorted by destination and padded so each node owns a
    fixed-width slot run inside a 512-slot tile (identical tile structure on
    all 8 cores; only index data differs - the program is pure SPMD).
"""

import os
from dataclasses import dataclass, field

import numpy as np

import concourse.bass as bass
import concourse.mybir as mybir
import concourse.bacc as bacc
import concourse.tile as tile
from concourse import bass_utils, bass2jax
from concourse.bass import ds

F32 = mybir.dt.float32
BF16 = mybir.dt.bfloat16
I16 = mybir.dt.int16

NCORES = 8
TSLOT = 512          # edge-slots per tile (== max matmul moving free dim)
SUPER = 4            # tiles per dma_gather call


# ----------------------------------------------------------------------------
# host-side preprocessing
# ----------------------------------------------------------------------------

@dataclass
class Plan:
    n: int
    npc: int
    tiles: list  # list of (D, n_t, pos0)  shared by all cores
    S: int       # total slots = TSLOT * len(tiles)
    perm: np.ndarray      # new position -> old node id
    vidx: list = field(default_factory=list)   # per-core wrapped [128, S/16] i16
    uidx: list = field(default_factory=list)
    has_iso: bool = False  # any zero-degree node anywhere


def make_plan(n: int, edge_index: np.ndarray) -> Plan:
    assert n % NCORES == 0
    npc = n // NCORES
    src = np.asarray(edge_index[0], dtype=np.int64)
    dst = np.asarray(edge_index[1], dtype=np.int64)
    deg = np.bincount(dst, minlength=n)

    # per-core block, degree-sorted (desc) within block
    perm = np.concatenate(
        [c * npc + np.argsort(-deg[c * npc:(c + 1) * npc], kind="stable")
         for c in range(NCORES)]
    )
    inv = np.empty(n, np.int64)
    inv[perm] = np.arange(n)
    src_n = inv[src]
    dst_n = inv[dst]
    deg_n = deg[perm]

    # shared tile structure from the max degree profile across cores
    degm = deg_n.reshape(NCORES, npc)
    maxdeg = degm.max(axis=0)
    tiles = []
    pos = 0
    while pos < npc:
        d = int(maxdeg[pos])
        d = max(2, d + (d & 1))          # even, >= 2
        n_t = min(TSLOT // d, npc - pos)
        tiles.append((d, n_t, pos))
        pos += n_t
    S = TSLOT * len(tiles)

    plan = Plan(n=n, npc=npc, tiles=tiles, S=S, perm=perm)
    plan.has_iso = bool((deg == 0).any())

    # per-core slot fill
    order = np.argsort(dst_n, kind="stable")
    src_s = src_n[order]
    dst_s = dst_n[order]
    starts = np.searchsorted(dst_s, np.arange(n))       # per new-id start
    for c in range(NCORES):
        dloc = deg_n[c * npc:(c + 1) * npc]
        vfill = np.full(npc, n, np.int64)      # sentinel: zero row
        ufill = np.full(npc, npc, np.int64)
        nz = dloc > 0
        gids = c * npc + np.arange(npc)
        vfill[nz] = src_s[starts[gids[nz]]]    # first in-edge's src
        ufill[nz] = np.arange(npc)[nz]

        vidx = np.full(S, n, np.int64)
        uidx = np.full(S, npc, np.int64)
        base_pos = np.empty(npc, np.int64)
        for ti, (d, n_t, pos0) in enumerate(tiles):
            sl0 = ti * TSLOT
            p = np.arange(pos0, pos0 + n_t)
            base_pos[p] = sl0 + (p - pos0) * d
            vidx[sl0:sl0 + n_t * d] = np.repeat(vfill[p], d)
            uidx[sl0:sl0 + n_t * d] = np.repeat(ufill[p], d)
        # overwrite real edges
        m = (dst_s >= c * npc) & (dst_s < (c + 1) * npc)
        es, ed = src_s[m], dst_s[m] - c * npc
        # rank within node: edges of a node are contiguous since sorted by dst
        rank = np.arange(len(ed)) - np.searchsorted(ed, ed)
        slots = base_pos[ed] + rank
        vidx[slots] = es
        uidx[slots] = ed

        def wrap(a):
            w = a.astype(np.int16).reshape(-1, 16).T   # [16, S/16]
            return np.tile(w, (8, 1)).copy()           # [128, S/16]
        plan.vidx.append(wrap(vidx))
        plan.uidx.append(wrap(uidx))
    return plan


def prep_inputs(inputs: dict, plan: Plan) -> list:
    """Build per-core in_maps (keys = dram tensor names)."""
    n, npc, perm = plan.n, plan.npc, plan.perm
    f32 = np.float32
    bf16 = np.dtype("bfloat16") if hasattr(np, "bfloat16") else None
    import ml_dtypes
    bf16 = ml_dtypes.bfloat16

    x = np.asarray(inputs["x"], f32)[perm]              # [n, 3] permuted
    deg = np.bincount(np.asarray(inputs["edge_index"][1]), minlength=n)
    mask = (deg[perm] > 0).astype(f32)                  # new order

    def lin(pref):
        wa = np.asarray(inputs[f"w{pref}a"], f32)
        ba = np.asarray(inputs[f"b{pref}a"], f32)
        wb = np.asarray(inputs[f"w{pref}b"], f32)
        bb = np.asarray(inputs[f"b{pref}b"], f32)
        return wa, ba, wb, bb

    w1a, b1a, w1b, b1b = lin("1")
    w2a, b2a, w2b, b2b = lin("2")
    w3a, b3a, w3b, b3b = lin("3")
    wm1 = np.asarray(inputs["wm1"], f32); bm1 = np.asarray(inputs["bm1"], f32)
    wm2 = np.asarray(inputs["wm2"], f32); bm2 = np.asarray(inputs["bm2"], f32)
    wm3 = np.asarray(inputs["wm3"], f32); bm3 = np.asarray(inputs["bm3"], f32)

    # v2: per-layer split weights  Wd = Wa[:cin]-Wa[cin:],  Wb = Wa[cin:]
    # edge tables hold raw h rows (128-padded); both halves of the first
    # linear run on the PE per edge-tile.
    def wsplit(wa, cin, cmid, cp):
        wd = np.zeros((cin, cp), f32); wb = np.zeros((128, cp), f32)
        wd[:, :cmid] = wa[:cin] - wa[cin:]
        wb[:cin, :cmid] = wa[cin:]
        return wd, wb
    wd1, wb1 = wsplit(w1a, 3, 64, 128)
    wd2, wb2 = wsplit(w2a, 64, 128, 128)
    wd3, wb3 = wsplit(w3a, 128, 256, 256)

    # edge matmul weights (second linear), padded, bf16
    we1 = np.zeros((128, 128), f32); we1[0:64, 0:64] = w1b
    we2 = w2b.astype(f32)
    we3 = w3b.reshape(2, 128, 256).astype(f32)          # [k, 128, 256]

    ba1 = np.zeros((128, 1), f32); ba1[0:64, 0] = b1a
    ba2 = b2a.reshape(128, 1).astype(f32)
    ba3 = b3a.reshape(2, 128).T.astype(f32)             # [128, 2]

    # L1 gather table: x rows padded to 128 cols (+ zero sentinel row)
    xtab = np.zeros((n + 1, 128), f32)
    xtab[:n, 0:3] = x
    xtab = xtab.astype(bf16)

    # bmask_l [128, J, npc] = bb[c] * mask[n]
    def bmask(bb, cmid, j, mloc):
        bpad = np.zeros(128 * j, f32)
        bpad[:cmid] = bb
        out = bpad.reshape(j, 128).transpose(1, 0)[:, :, None] * mloc[None, None, :]
        return np.ascontiguousarray(out, dtype=bf16)

    # head weights: rearrange wm1 rows to hstack layout [h1(64) 0(64) h2 h3]
    wm1_arr = np.zeros((512, 512), f32)
    wm1_arr[0:64] = wm1[0:64]
    wm1_arr[128:256] = wm1[64:192]
    wm1_arr[256:512] = wm1[192:448]

    in_maps = []
    for c in range(NCORES):
        mloc = mask[c * npc:(c + 1) * npc]
        m = {
            "xT": np.ascontiguousarray(x[c * npc:(c + 1) * npc].T),   # [3, npc]
            "xtab": xtab,
            "vidx": plan.vidx[c],
            "wd1": wd1, "wd2": wd2.astype(bf16), "wd3": wd3.astype(bf16),
            "wb1": wb1.astype(bf16), "wb2": wb2.astype(bf16),
            "wb3": wb3.astype(bf16),
            "we1": we1.astype(bf16), "we2": we2.astype(bf16),
            "we3": np.ascontiguousarray(we3.transpose(1, 0, 2)).astype(bf16),
            "ident": np.eye(128, dtype=np.float32).astype(bf16),
            "ba1": ba1, "ba2": ba2, "ba3": ba3,
            "bm1": bmask(b1b, 64, 1, mloc),
            "bm2": bmask(b2b, 128, 1, mloc),
            "bm3": bmask(b3b, 256, 2, mloc),
            "wh1": np.ascontiguousarray(
                wm1_arr.reshape(4, 128, 512).transpose(1, 0, 2)).astype(bf16),
            "wh2": np.ascontiguousarray(
                wm2.reshape(4, 128, 256).transpose(1, 0, 2)).astype(bf16),
            "wh3": np.ascontiguousarray(
                wm3.reshape(2, 128, 4).transpose(1, 0, 2)).astype(bf16),
            "bh1": np.ascontiguousarray(bm1.reshape(4, 128).T),
            "bh2": np.ascontiguousarray(bm2.reshape(2, 128).T),
            "bh3": bm3.reshape(4, 1).astype(f32),
        }
        in_maps.append(m)
    return in_maps


# ----------------------------------------------------------------------------
# device program
# ----------------------------------------------------------------------------

LAYERS = [
    # (name, C_in, C_mid(padded J*128), J, hs_lhs(prev h chunk), hs_out(j->chunk))
    dict(name="1", cin=3, cp=128, j=1, out_chunks=[0]),
    dict(name="2", cin=64, cp=128, j=1, out_chunks=[1]),
    dict(name="3", cin=128, cp=256, j=2, out_chunks=[2, 3]),
]


def build_program(plan: Plan, nlayers: int = 3, with_head: bool = True,
                  with_edge: bool = True, timeline: bool = False):
    n, npc, S = plan.n, plan.npc, plan.S
    ntiles = len(plan.tiles)
    nc = bacc.Bacc(
        "TRN2", target_bir_lowering=False, debug=False,
        enable_asserts=False, num_devices=1 if timeline else NCORES,
    )
    RG = [list(range(NCORES))]

    # ---- dram tensors -------------------------------------------------------
    din = {}
    def dram_in(name, shape, dt):
        din[name] = nc.dram_tensor(name, list(shape), dt, kind="ExternalInput")
        return din[name]

    xT = dram_in("xT", (3, npc), F32)
    xtab_d = dram_in("xtab", (n + 1, 128), BF16)
    vidx_d = dram_in("vidx", (128, S // 16), I16)
    wd_d = [dram_in("wd1", (3, 128), F32), dram_in("wd2", (64, 128), BF16),
            dram_in("wd3", (128, 256), BF16)]
    wb_d = [dram_in("wb1", (128, 128), BF16), dram_in("wb2", (128, 128), BF16),
            dram_in("wb3", (128, 256), BF16)]
    we_d = [dram_in("we1", (128, 128), BF16), dram_in("we2", (128, 128), BF16),
            dram_in("we3", (128, 2, 256), BF16)]
    ident_d = dram_in("ident", (128, 128), BF16)
    ba_d = [dram_in("ba1", (128, 1), F32), dram_in("ba2", (128, 1), F32),
            dram_in("ba3", (128, 2), F32)]
    bm_d = [dram_in("bm1", (128, 1, npc), BF16),
            dram_in("bm2", (128, 1, npc), BF16),
            dram_in("bm3", (128, 2, npc), BF16)]
    wh_d = [dram_in("wh1", (128, 4, 512), BF16),
            dram_in("wh2", (128, 4, 256), BF16),
            dram_in("wh3", (128, 2, 4), BF16)]
    bh_d = [dram_in("bh1", (128, 4), F32), dram_in("bh2", (128, 2), F32),
            dram_in("bh3", (4, 1), F32)]
    outT = nc.dram_tensor("outT", [4, npc], F32, kind="ExternalOutput")

    # internal row tables of h_(l-1) for layers 2,3 (l=1 uses xtab input).
    # NOTE: dma_gather cannot read Shared-addr-space scratchpad; keep Local.
    hag_t = [None] + [nc.dram_tensor(f"hag{i}", [npc, 128], BF16,
                                     kind="Internal") for i in (2, 3)]
    tab_t = [None] + [nc.dram_tensor(f"tab{i}", [n + 1, 128], BF16,
                                     kind="Internal") for i in (2, 3)]

    with tile.TileContext(nc) as tc:
        with (
            tc.tile_pool(name="singles", bufs=1) as sing,
            tc.tile_pool(name="stage", bufs=3) as stg,
            tc.tile_pool(name="gather", bufs=2) as gat,
            tc.tile_pool(name="edge", bufs=3) as edg,
        ):
            # ---- load constants into SBUF ----
            def load(dt_handle, shape, dtype, tag):
                t = sing.tile(list(shape), dtype, tag=tag)
                nc.sync.dma_start(t, dt_handle[...])
                return t

            xT_s = load(xT, (3, npc), F32, "xT")
            vidx_s = load(vidx_d, (128, S // 16), I16, "vidx")
            wd_s = [load(wd_d[0], (3, 128), F32, "wd0"),
                    load(wd_d[1], (64, 128), BF16, "wd1"),
                    load(wd_d[2], (128, 256), BF16, "wd2")]
            wb_s = [load(wb_d[i], wb_d[i].shape, BF16, f"wb{i}")
                    for i in range(3)]
            ident = load(ident_d, (128, 128), BF16, "ident")
            we_s = [load(we_d[i], we_d[i].shape, BF16, f"we{i}")
                    for i in range(3)]
            ba_s = [load(ba_d[i], ba_d[i].shape, F32, f"ba{i}")
                    for i in range(3)]
            bm_s = [load(bm_d[i], bm_d[i].shape, BF16, f"bm{i}")
                    for i in range(3)]
            wh_s = [load(wh_d[i], wh_d[i].shape, BF16, f"wh{i}")
                    for i in range(3)]
            bh_s = [load(bh_d[i], bh_d[i].shape, F32, f"bh{i}")
                    for i in range(3)]

            hstack = sing.tile([128, 4, npc], BF16)     # [h1;0 | h2 | h3a | h3b]
            if not with_edge:
                nc.vector.memset(hstack, 0.0)
            zrow = sing.tile([1, 512], BF16)
            nc.vector.memset(zrow, 0.0)

            # =============== per layer ===============
            from contextlib import ExitStack
            _ps_stack = ExitStack()
            psp = _ps_stack.enter_context(
                tc.tile_pool(name="psum", bufs=2, space="PSUM"))
            for li, L in enumerate(LAYERS[:nlayers]):
                cin, cp, J = L["cin"], L["cp"], L["j"]

                # ---- build row table of h_(l-1) (PE transpose) + AG ----
                if li > 0:
                    nchunks = (npc + 127) // 128
                    for cn in range(nchunks):
                        r0 = cn * 128
                        m = min(128, npc - r0)
                        if li == 1:
                            lhsT = hstack[0:64, 0, ds(r0, m)]
                            kdim = 64
                        else:
                            lhsT = hstack[:, 1, ds(r0, m)]
                            kdim = 128
                        ps = psp.tile([128, 128], F32, tag="edge_ps")
                        nc.tensor.matmul(ps[:m], lhsT, ident[:kdim, :],
                                         start=True, stop=True)
                        st = stg.tile([128, 128], BF16, tag="uv_stage")
                        nc.scalar.copy(st[:m], ps[:m])
                        nc.sync.dma_start(hag_t[li][ds(r0, m), :], st[:m])
                    if timeline:
                        nc.sync.dma_start(tab_t[li][0:npc, :], hag_t[li][...])
                    else:
                        nc.gpsimd.collective_compute(
                            "AllGather", mybir.AluOpType.bypass, RG,
                            ins=[hag_t[li][...]], outs=[tab_t[li][0:n, :]],
                        )
                    nc.sync.dma_start(tab_t[li][n:n + 1, :], zrow[:, 0:128])
                tab = xtab_d if li == 0 else tab_t[li]

                # ---- edge phase ----
                ba = ba_s[li]
                hprev = (xT_s if li == 0 else
                         hstack[0:64, 0, :] if li == 1 else hstack[:, 1, :])
                wd = wd_s[li]
                for st0 in (range(0, ntiles, SUPER) if with_edge else []):
                    g = min(SUPER, ntiles - st0)
                    nidx = g * TSLOT
                    vg = gat.tile([128, 1, nidx], BF16, tag="vg")
                    c0 = st0 * TSLOT // 16
                    nc.gpsimd.dma_gather(
                        vg, tab[...],
                        vidx_s[:, ds(c0, nidx // 16)],
                        nidx, nidx, elem_size=128, transpose=True,
                        single_packet=False)
                    for tt in range(g):
                        d, n_t, pos0 = plan.tiles[st0 + tt]
                        T = n_t * d
                        o = tt * TSLOT
                        # pre-activation: psum = Wd^T h_dst(bcast) + Wb^T h_src
                        pp = psp.tile([128, J, TSLOT], F32, tag="pre_ps2")
                        hb = hprev[:, ds(pos0, n_t)].rearrange(
                            "p (nt one) -> p nt one", one=1).broadcast_to(
                                (hprev.shape[0], n_t, d))
                        for jj in range(J):
                            nc.tensor.matmul(
                                pp[:, jj, 0:T], wd[:, ds(jj * 128, 128)], hb,
                                start=True, stop=False)
                            nc.tensor.matmul(
                                pp[:, jj, 0:T], wb_s[li][:, ds(jj * 128, 128)],
                                vg[:, 0, ds(o, T)],
                                start=False, stop=True)
                        # relu + first bias, evict to bf16
                        rl = edg.tile([128, J, TSLOT], BF16, tag="rl")
                        for jj in range(J):
                            nc.scalar.activation(
                                rl[:, jj, 0:T], pp[:, jj, 0:T],
                                mybir.ActivationFunctionType.Relu,
                                bias=ba[:, jj:jj + 1])
                        ps = psp.tile([128, J, TSLOT], F32, tag="edge_ps")
                        for jj in range(J):
                            for kk in range(J):
                                w = (we_s[li][...] if J == 1
                                     else we_s[li][:, kk, ds(jj * 128, 128)])
                                nc.tensor.matmul(
                                    ps[:, jj, 0:T], w, rl[:, kk, 0:T],
                                    start=(kk == 0), stop=(kk == J - 1))
                        tmp = edg.tile([128, J, 128], BF16, tag="agg")
                        for jj in range(J):
                            nc.vector.reduce_max(
                                tmp[:, jj, 0:n_t],
                                ps[:, jj, 0:T].rearrange(
                                    "p (nt d) -> p nt d", d=d),
                                axis=mybir.AxisListType.X)
                        for jj in range(J):
                            nc.vector.tensor_add(
                                hstack[:, L["out_chunks"][jj], ds(pos0, n_t)],
                                tmp[:, jj, 0:n_t],
                                bm_s[li][:, jj, ds(pos0, n_t)])

            # =============== head ===============
            _ps_stack.close()
            with tc.tile_pool(name="psum1", bufs=1, space="PSUM") as psp1:
                outsb = sing.tile([4, npc], F32)
                if not with_head:
                    nc.vector.tensor_copy(outsb, hstack[0:4, 0, :])
                    nc.sync.dma_start(outT[...], outsb)
                for c0 in (range(0, npc, 512) if with_head else []):
                    w = min(512, npc - c0)
                    ps1 = psp1.tile([128, 4, 512], F32, tag="h_ps1")
                    for jj in range(4):
                        for kk in range(4):
                            nc.tensor.matmul(
                                ps1[:, jj, 0:w], wh_s[0][:, kk, ds(jj * 128, 128)],
                                hstack[:, kk, ds(c0, w)],
                                start=(kk == 0), stop=(kk == 3))
                    m1 = edg.tile([128, 4, 512], BF16, tag="h_m1")
                    for jj in range(4):
                        nc.scalar.activation(
                            m1[:, jj, 0:w], ps1[:, jj, 0:w],
                            mybir.ActivationFunctionType.Relu,
                            bias=bh_s[0][:, jj:jj + 1])
                    ps2 = psp1.tile([128, 2, 512], F32, tag="h_ps2")
                    for jj in range(2):
                        for kk in range(4):
                            nc.tensor.matmul(
                                ps2[:, jj, 0:w], wh_s[1][:, kk, ds(jj * 128, 128)],
                                m1[:, kk, 0:w],
                                start=(kk == 0), stop=(kk == 3))
                    m2 = edg.tile([128, 2, 512], BF16, tag="h_m2")
                    for jj in range(2):
                        nc.scalar.activation(
                            m2[:, jj, 0:w], ps2[:, jj, 0:w],
                            mybir.ActivationFunctionType.Relu,
                            bias=bh_s[1][:, jj:jj + 1])
                    ps3 = psp1.tile([4, 512], F32, tag="h_ps3")
                    for kk in range(2):
                        nc.tensor.matmul(ps3[:, 0:w], wh_s[2][:, kk, :],
                                         m2[:, kk, 0:w],
                                         start=(kk == 0), stop=(kk == 1))
                    nc.scalar.activation(
                        outsb[:, ds(c0, w)], ps3[:, 0:w],
                        mybir.ActivationFunctionType.Identity,
                        bias=bh_s[2][:, 0:1])
                if with_head:
                    nc.sync.dma_start(outT[...], outsb)

    nc.compile()
    return nc


# ----------------------------------------------------------------------------
# entry point
# ----------------------------------------------------------------------------

class Runner:
    """Compile once; keep the jitted sharded executable for cheap re-runs."""

    def __init__(self, nc):
        import jax
        from jax.sharding import Mesh, PartitionSpec
        from jax.experimental.shard_map import shard_map

        bass2jax.install_neuronx_cc_hook()
        self.nc = nc
        partition_name = (nc.partition_id_tensor.name
                          if nc.partition_id_tensor else None)
        in_names, out_names, out_avals, zero_outs = [], [], [], []
        for alloc in nc.m.functions[0].allocations:
            if not isinstance(alloc, mybir.MemoryLocationSet):
                continue
            name = alloc.memorylocations[0].name
            if alloc.kind == "ExternalInput":
                if name != partition_name:
                    in_names.append(name)
            elif alloc.kind == "ExternalOutput":
                shape = tuple(alloc.tensor_shape)
                dtype = mybir.dt.np(alloc.dtype)
                out_names.append(name)
                out_avals.append(jax.core.ShapedArray(shape, dtype))
                zero_outs.append(np.zeros(shape, dtype))
        n_params = len(in_names)
        all_in = in_names + out_names
        if partition_name is not None:
            all_in.append(partition_name)
        self.in_names = in_names
        self.out_names = out_names
        self.out_avals = out_avals
        self.zero_outs = zero_outs

        def _body(*args):
            operands = list(args)
            if partition_name is not None:
                operands.append(bass2jax.partition_id_tensor())
            return tuple(bass2jax._bass_exec_p.bind(
                *operands, out_avals=tuple(out_avals),
                in_names=tuple(all_in), out_names=tuple(out_names),
                lowering_input_output_aliases=(),
                sim_require_finite=True, sim_require_nnan=True, nc=nc))

        devices = jax.devices()[:NCORES]
        mesh = Mesh(np.asarray(devices), ("core",))
        self.mesh = mesh
        n_outs = len(out_names)
        self.sharded = jax.jit(
            shard_map(_body, mesh=mesh,
                      in_specs=(PartitionSpec("core"),) * (n_params + n_outs),
                      out_specs=(PartitionSpec("core"),) * n_outs,
                      check_rep=False),
            keep_unused=True)

    def prepare(self, in_maps):
        """Upload per-core inputs to the devices once; returns dev args."""
        import jax
        from jax.sharding import NamedSharding, PartitionSpec
        sh = NamedSharding(self.mesh, PartitionSpec("core"))
        concat_in = [
            np.concatenate([np.asarray(in_maps[c][nm])
                            for c in range(NCORES)], axis=0)
            for nm in self.in_names]
        concat_zeros = [np.zeros((NCORES * z.shape[0], *z.shape[1:]), z.dtype)
                        for z in self.zero_outs]
        args = [jax.device_put(a, sh) for a in concat_in + concat_zeros]
        jax.block_until_ready(args)
        return args

    def run(self, dev_args):
        import jax
        outs = self.sharded(*dev_args)
        jax.block_until_ready(outs)
        return outs

    def __call__(self, in_maps):
        outs = self.run(self.prepare(in_maps))
        return [
            {nm: np.asarray(outs[i]).reshape(
                NCORES, *self.out_avals[i].shape)[c]
             for i, nm in enumerate(self.out_names)}
            for c in range(NCORES)]


_CACHE = {}


def get_compiled(inputs: dict):
    x = np.asarray(inputs["x"])
    ei = np.asarray(inputs["edge_index"])
    n = x.shape[0]
    key = (n, ei.shape[1], hash(ei.tobytes()))
    if key not in _CACHE:
        plan = make_plan(n, ei)
        nc = build_program(plan)
        _CACHE.clear()
        _CACHE[key] = (plan, Runner(nc))
    return _CACHE[key]


def _run(inputs: dict) -> np.ndarray:
    plan, runner = get_compiled(inputs)
    in_maps = prep_inputs(inputs, plan)
    results = runner(in_maps)
    npc, n = plan.npc, plan.n
    out = np.empty((n, 4), np.float32)
    for c in range(NCORES):
        out[plan.perm[c * npc:(c + 1) * npc]] = results[c]["outT"].T
    return out


def kernel(**inputs) -> np.ndarray:
    return _run(inputs)



# revision 11
# speedup vs baseline: 50.3344x; 50.3344x over previous
"""DGCNN segmentation (3x EdgeConv max-aggregation + MLP head) on 8 Trainium2 cores.

Sharding: nodes are split into 8 equal contiguous blocks (one per core); each
core owns all edges whose *destination* lies in its block, so the scatter-max
aggregation is core-local.  Per-layer node tables are AllGather'd in 5
row-groups (overlapped with the producing edge phase) so every core can gather
any source node's row.

Per layer l (C_in -> C -> C, PyG EdgeConv):
    m_e   = relu(u[dst_e] + v[src_e] + ba_l) @ Wb_l          (per edge)
    h_i   = max_{e: dst_e = i} m_e + bb_l   (0 if no edges)
  where u = h @ (Wa_l[:C_in] - Wa_l[C_in:]),  v = h @ Wa_l[C_in:].

Device pipeline per core:
  - node row-tables in HBM (bf16, grouped layout); per-edge transposed gather
    via gpsimd.dma_gather(transpose=True) puts channels on partitions.
    Gathers round-robin over 4 SWDGE queues (4 descriptor rings drain in
    parallel: ~4x the single-ring gather bandwidth).
  - PE matmuls compute Wd^T h_dst (broadcast over each node's slot run) +
    Wb^T h_src into PSUM, scalar-engine relu+bias, PE matmul with We, then a
    segmented max over each node's padded slot-block on the vector engine.
  - Edges are pre-sorted by destination and padded so each node owns a
    fixed-width slot run inside a 512-slot tile (identical tile structure on
    all 8 cores; only index data differs - the program is pure SPMD).
"""

import os
from dataclasses import dataclass, field

import numpy as np

import concourse.bass as bass
import concourse.mybir as mybir
import concourse.bacc as bacc
import concourse.tile as tile
from concourse import bass_utils, bass2jax
from concourse.bass import ds

F32 = mybir.dt.float32
BF16 = mybir.dt.bfloat16
I16 = mybir.dt.int16

NCORES = 8
TSLOT = 512          # edge-slots per tile (== max matmul moving free dim)
SUPER = 4            # tiles per dma_gather call
NQ = 1               # SWDGE queues used round-robin for gathers
NGROUP = 5           # table AllGather row-groups
GSZ = 768            # rows per group (npc padded to NGROUP*GSZ = 3840)


# ----------------------------------------------------------------------------
# host-side preprocessing
# ----------------------------------------------------------------------------

@dataclass
class Plan:
    n: int
    npc: int
    npc_pad: int          # NGROUP * GSZ
    ntab: int             # NCORES * npc_pad (+1 sentinel row at index ntab)
    tiles: list  # list of (D, n_t, pos0)  shared by all cores
    S: int       # total slots = TSLOT * len(tiles)
    perm: np.ndarray      # new position -> old node id
    vidx: list = field(default_factory=list)   # per-core wrapped [128, S/16] i16
    has_iso: bool = False  # any zero-degree node anywhere


def _remap_rows(pos, npc, npc_pad):
    """Map node position (c*npc + r) -> grouped table row id."""
    c, r = pos // npc, pos % npc
    g = r // GSZ
    return g * (NCORES * GSZ) + c * GSZ + (r - g * GSZ)


def make_plan(n: int, edge_index: np.ndarray) -> Plan:
    assert n % NCORES == 0
    npc = n // NCORES
    npc_pad = NGROUP * GSZ
    assert npc <= npc_pad
    ntab = NCORES * npc_pad
    src = np.asarray(edge_index[0], dtype=np.int64)
    dst = np.asarray(edge_index[1], dtype=np.int64)
    deg = np.bincount(dst, minlength=n)

    # per-core block, degree-sorted (desc) within block
    perm = np.concatenate(
        [c * npc + np.argsort(-deg[c * npc:(c + 1) * npc], kind="stable")
         for c in range(NCORES)]
    )
    inv = np.empty(n, np.int64)
    inv[perm] = np.arange(n)
    src_n = inv[src]
    dst_n = inv[dst]
    deg_n = deg[perm]

    # shared tile structure from the max degree profile across cores
    degm = deg_n.reshape(NCORES, npc)
    maxdeg = degm.max(axis=0)
    tiles = []
    pos = 0
    while pos < npc:
        d = int(maxdeg[pos])
        d = max(2, d + (d & 1))          # even, >= 2
        n_t = min(TSLOT // d, npc - pos)
        tiles.append((d, n_t, pos))
        pos += n_t
    S = TSLOT * len(tiles)

    plan = Plan(n=n, npc=npc, npc_pad=npc_pad, ntab=ntab, tiles=tiles, S=S,
                perm=perm)
    plan.has_iso = bool((deg == 0).any())

    # grouped-table row id for each node position
    rowid = _remap_rows(np.arange(n), npc, npc_pad)

    # per-core slot fill
    order = np.argsort(dst_n, kind="stable")
    src_s = src_n[order]
    dst_s = dst_n[order]
    starts = np.searchsorted(dst_s, np.arange(n))       # per new-id start
    for c in range(NCORES):
        dloc = deg_n[c * npc:(c + 1) * npc]
        vfill = np.full(npc, ntab, np.int64)   # sentinel: zero row
        nz = dloc > 0
        gids = c * npc + np.arange(npc)
        vfill[nz] = rowid[src_s[starts[gids[nz]]]]  # first in-edge's src row

        vidx = np.full(S, ntab, np.int64)
        base_pos = np.empty(npc, np.int64)
        for ti, (d, n_t, pos0) in enumerate(tiles):
            sl0 = ti * TSLOT
            p = np.arange(pos0, pos0 + n_t)
            base_pos[p] = sl0 + (p - pos0) * d
            vidx[sl0:sl0 + n_t * d] = np.repeat(vfill[p], d)
        # overwrite real edges
        m = (dst_s >= c * npc) & (dst_s < (c + 1) * npc)
        es, ed = src_s[m], dst_s[m] - c * npc
        # rank within node: edges of a node are contiguous since sorted by dst
        rank = np.arange(len(ed)) - np.searchsorted(ed, ed)
        slots = base_pos[ed] + rank
        vidx[slots] = rowid[es]

        def wrap(a):
            w = a.astype(np.int16).reshape(-1, 16).T   # [16, S/16]
            return np.tile(w, (8, 1)).copy()           # [128, S/16]
        plan.vidx.append(wrap(vidx))
    return plan


def prep_inputs(inputs: dict, plan: Plan) -> list:
    """Build per-core in_maps (keys = dram tensor names)."""
    n, npc, perm = plan.n, plan.npc, plan.perm
    f32 = np.float32
    import ml_dtypes
    bf16 = ml_dtypes.bfloat16

    x = np.asarray(inputs["x"], f32)[perm]              # [n, 3] permuted
    deg = np.bincount(np.asarray(inputs["edge_index"][1]), minlength=n)
    mask = (deg[perm] > 0).astype(f32)                  # new order

    def lin(pref):
        wa = np.asarray(inputs[f"w{pref}a"], f32)
        ba = np.asarray(inputs[f"b{pref}a"], f32)
        wb = np.asarray(inputs[f"w{pref}b"], f32)
        bb = np.asarray(inputs[f"b{pref}b"], f32)
        return wa, ba, wb, bb

    w1a, b1a, w1b, b1b = lin("1")
    w2a, b2a, w2b, b2b = lin("2")
    w3a, b3a, w3b, b3b = lin("3")
    wm1 = np.asarray(inputs["wm1"], f32); bm1 = np.asarray(inputs["bm1"], f32)
    wm2 = np.asarray(inputs["wm2"], f32); bm2 = np.asarray(inputs["bm2"], f32)
    wm3 = np.asarray(inputs["wm3"], f32); bm3 = np.asarray(inputs["bm3"], f32)

    # per-layer split weights  Wd = Wa[:cin]-Wa[cin:],  Wb = Wa[cin:]
    # edge tables hold raw h rows (128-padded); both halves of the first
    # linear run on the PE per edge-tile.
    def wsplit(wa, cin, cmid, cp):
        wd = np.zeros((cin, cp), f32); wb = np.zeros((128, cp), f32)
        wd[:, :cmid] = wa[:cin] - wa[cin:]
        wb[:cin, :cmid] = wa[cin:]
        return wd, wb
    wd1, wb1 = wsplit(w1a, 3, 64, 128)
    wd2, wb2 = wsplit(w2a, 64, 128, 128)
    wd3, wb3 = wsplit(w3a, 128, 256, 256)

    # edge matmul weights (second linear), padded, bf16
    we1 = np.zeros((128, 128), f32); we1[0:64, 0:64] = w1b
    we2 = w2b.astype(f32)
    we3 = w3b.reshape(2, 128, 256).astype(f32)          # [k, 128, 256]

    ba1 = np.zeros((128, 1), f32); ba1[0:64, 0] = b1a
    ba2 = b2a.reshape(128, 1).astype(f32)
    ba3 = b3a.reshape(2, 128).T.astype(f32)             # [128, 2]

    # L1 gather table: x rows padded to 128 cols, grouped row layout
    xtab = np.zeros((plan.ntab + 1, 128), f32)
    rowid = _remap_rows(np.arange(n), npc, plan.npc_pad)
    xtab[rowid, 0:3] = x
    xtab = xtab.astype(bf16)

    # bmask_l [128, J, npc] = bb[c] * mask[n]
    def bmask(bb, cmid, j, mloc):
        bpad = np.zeros(128 * j, f32)
        bpad[:cmid] = bb
        out = bpad.reshape(j, 128).transpose(1, 0)[:, :, None] * mloc[None, None, :]
        return np.ascontiguousarray(out, dtype=bf16)

    # head weights: rearrange wm1 rows to hstack layout [h1(64) 0(64) h2 h3]
    wm1_arr = np.zeros((512, 512), f32)
    wm1_arr[0:64] = wm1[0:64]
    wm1_arr[128:256] = wm1[64:192]
    wm1_arr[256:512] = wm1[192:448]

    in_maps = []
    for c in range(NCORES):
        mloc = mask[c * npc:(c + 1) * npc]
        m = {
            "xT": np.ascontiguousarray(
                x[c * npc:(c + 1) * npc].T).astype(bf16),   # [3, npc]
            "xtab": xtab,
            "vidx": plan.vidx[c],
            "wd1": wd1.astype(bf16), "wd2": wd2.astype(bf16),
            "wd3": wd3.astype(bf16),
            "wb1": wb1.astype(bf16), "wb2": wb2.astype(bf16),
            "wb3": wb3.astype(bf16),
            "we1": we1.astype(bf16), "we2": we2.astype(bf16),
            "we3": np.ascontiguousarray(we3.transpose(1, 0, 2)).astype(bf16),
            "ident": np.eye(128, dtype=np.float32).astype(bf16),
            "ba1": ba1, "ba2": ba2, "ba3": ba3,
            "bm1": bmask(b1b, 64, 1, mloc),
            "bm2": bmask(b2b, 128, 1, mloc),
            "bm3": bmask(b3b, 256, 2, mloc),
            "wh1": np.ascontiguousarray(
                wm1_arr.reshape(4, 128, 512).transpose(1, 0, 2)).astype(bf16),
            "wh2": np.ascontiguousarray(
                wm2.reshape(4, 128, 256).transpose(1, 0, 2)).astype(bf16),
            "wh3": np.ascontiguousarray(
                wm3.reshape(2, 128, 4).transpose(1, 0, 2)).astype(bf16),
            "bh1": np.ascontiguousarray(bm1.reshape(4, 128).T),
            "bh2": np.ascontiguousarray(bm2.reshape(2, 128).T),
            "bh3": bm3.reshape(4, 1).astype(f32),
        }
        in_maps.append(m)
    return in_maps


# ----------------------------------------------------------------------------
# device program
# ----------------------------------------------------------------------------

LAYERS = [
    # (name, C_in, C_mid(padded J*128), J, hs_lhs(prev h chunk), hs_out(j->chunk))
    dict(name="1", cin=3, cp=128, j=1, out_chunks=[0]),
    dict(name="2", cin=64, cp=128, j=1, out_chunks=[1]),
    dict(name="3", cin=128, cp=256, j=2, out_chunks=[2, 3]),
]


def build_program(plan: Plan, nlayers: int = 3, with_head: bool = True,
                  with_edge: bool = True, timeline: bool = False):
    n, npc, S = plan.n, plan.npc, plan.S
    npc_pad, ntab = plan.npc_pad, plan.ntab
    ntiles = len(plan.tiles)
    nc = bacc.Bacc(
        "TRN2", target_bir_lowering=False, debug=False,
        enable_asserts=False, num_devices=1 if timeline else NCORES,
        num_swdge_queues=NQ,
    )
    RG = [list(range(NCORES))]

    # ---- dram tensors -------------------------------------------------------
    din = {}
    def dram_in(name, shape, dt):
        din[name] = nc.dram_tensor(name, list(shape), dt, kind="ExternalInput")
        return din[name]

    xT = dram_in("xT", (3, npc), BF16)
    xtab_d = dram_in("xtab", (ntab + 1, 128), BF16)
    vidx_d = dram_in("vidx", (128, S // 16), I16)
    wd_d = [dram_in("wd1", (3, 128), BF16), dram_in("wd2", (64, 128), BF16),
            dram_in("wd3", (128, 256), BF16)]
    wb_d = [dram_in("wb1", (128, 128), BF16), dram_in("wb2", (128, 128), BF16),
            dram_in("wb3", (128, 256), BF16)]
    we_d = [dram_in("we1", (128, 128), BF16), dram_in("we2", (128, 128), BF16),
            dram_in("we3", (128, 2, 256), BF16)]
    ident_d = dram_in("ident", (128, 128), BF16)
    ba_d = [dram_in("ba1", (128, 1), F32), dram_in("ba2", (128, 1), F32),
            dram_in("ba3", (128, 2), F32)]
    bm_d = [dram_in("bm1", (128, 1, npc), BF16),
            dram_in("bm2", (128, 1, npc), BF16),
            dram_in("bm3", (128, 2, npc), BF16)]
    wh_d = [dram_in("wh1", (128, 4, 512), BF16),
            dram_in("wh2", (128, 4, 256), BF16),
            dram_in("wh3", (128, 2, 4), BF16)]
    bh_d = [dram_in("bh1", (128, 4), F32), dram_in("bh2", (128, 2), F32),
            dram_in("bh3", (4, 1), F32)]
    outT = nc.dram_tensor("outT", [4, npc], F32, kind="ExternalOutput")

    # internal row tables of h_(l-1) for layers 2,3 (l=1 uses xtab input).
    # NOTE: dma_gather cannot read Shared-addr-space scratchpad; keep Local.
    # One hag tensor per AllGather row-group: each collective then reads a
    # full tensor (whole-tensor dep tracking; sliced collective ins raced).
    hag_t = [None] + [[nc.dram_tensor(f"hag{i}g{g}", [GSZ, 128], BF16,
                                      kind="Internal") for g in range(NGROUP)]
                      for i in (2, 3)]
    tab_t = [None] + [nc.dram_tensor(f"tab{i}", [ntab + 1, 128], BF16,
                                     kind="Internal") for i in (2, 3)]

    with tile.TileContext(nc) as tc:
        with (
            tc.tile_pool(name="singles", bufs=1) as sing,
            tc.tile_pool(name="stage", bufs=3) as stg,
            tc.tile_pool(name="gather", bufs=6) as gat,
            tc.tile_pool(name="edge", bufs=3) as edg,
        ):
            # ---- load constants into SBUF ----
            def load(dt_handle, shape, dtype, tag):
                t = sing.tile(list(shape), dtype, tag=tag)
                nc.sync.dma_start(t, dt_handle[...])
                return t

            xT_s = load(xT, (3, npc), BF16, "xT")
            vidx_s = load(vidx_d, (128, S // 16), I16, "vidx")
            wd_s = [load(wd_d[0], (3, 128), BF16, "wd0"),
                    load(wd_d[1], (64, 128), BF16, "wd1"),
                    load(wd_d[2], (128, 256), BF16, "wd2")]
            wb_s = [load(wb_d[i], wb_d[i].shape, BF16, f"wb{i}")
                    for i in range(3)]
            ident = load(ident_d, (128, 128), BF16, "ident")
            we_s = [load(we_d[i], we_d[i].shape, BF16, f"we{i}")
                    for i in range(3)]
            ba_s = [load(ba_d[i], ba_d[i].shape, F32, f"ba{i}")
                    for i in range(3)]
            bm_s = [load(bm_d[i], bm_d[i].shape, BF16, f"bm{i}")
                    for i in range(3)]
            wh_s = [load(wh_d[i], wh_d[i].shape, BF16, f"wh{i}")
                    for i in range(3)]
            bh_s = [load(bh_d[i], bh_d[i].shape, F32, f"bh{i}")
                    for i in range(3)]

            hstack = sing.tile([128, 4, npc], BF16)     # [h1;0 | h2 | h3a | h3b]
            if not with_edge:
                nc.vector.memset(hstack, 0.0)
            zrow = sing.tile([1, 512], BF16, tag="zrow")
            nc.vector.memset(zrow, 0.0)
            ztile = sing.tile([128, 128], BF16, tag="ztile")
            nc.vector.memset(ztile, 0.0)

            # =============== per layer ===============
            from contextlib import ExitStack
            _ps_stack = ExitStack()
            psp = _ps_stack.enter_context(
                tc.tile_pool(name="psum", bufs=2, space="PSUM"))
            gq = [0]  # round-robin gather queue counter
            nchunks = (npc + 127) // 128

            # table production for layer li+1, interleaved into layer li's
            # edge loop: transpose 128-node chunks of this layer's output as
            # they aggregate, DMA to hag, and fire each row-group's AllGather
            # as soon as its chunks have landed.
            def emit_table_chunk(nli, cn):
                """Transpose+store hag chunk cn of the table for layer nli."""
                r0 = cn * 128
                m = min(128, npc - r0)
                grp = r0 // GSZ            # GSZ % 128 == 0: chunk in 1 group
                go = r0 - grp * GSZ
                if nli == 1:
                    lhsT = hstack[0:64, 0, ds(r0, m)]
                    kdim = 64
                else:
                    lhsT = hstack[:, 1, ds(r0, m)]
                    kdim = 128
                ps = psp.tile([128, 128], F32, tag="edge_ps")
                nc.tensor.matmul(ps[:m], lhsT, ident[:kdim, :],
                                 start=True, stop=True)
                st = stg.tile([128, 128], BF16, tag="uv_stage")
                nc.scalar.copy(st[:m], ps[:m])
                nc.sync.dma_start(hag_t[nli][grp][ds(go, m), :], st[:m])

            def emit_group_collective(nli, g):
                o0 = g * (NCORES * GSZ)
                if timeline:
                    nc.sync.dma_start(
                        tab_t[nli][ds(o0, GSZ), :], hag_t[nli][g][...])
                else:
                    nc.gpsimd.collective_compute(
                        "AllGather", mybir.AluOpType.bypass, RG,
                        ins=[hag_t[nli][g][...]],
                        outs=[tab_t[nli][ds(o0, NCORES * GSZ), :]],
                    )

            for li, L in enumerate(LAYERS[:nlayers]):
                cin, cp, J = L["cin"], L["cp"], L["j"]
                tab = xtab_d if li == 0 else tab_t[li]

                # next layer's table bookkeeping
                nli = li + 1
                produce = with_edge and nli < nlayers and nli > 0
                if produce:
                    # zero padded tail rows [npc, npc_pad) once, up front
                    # (rows 678..768 of the last group's hag tensor)
                    toff = npc - (NGROUP - 1) * GSZ
                    nc.sync.dma_start(
                        hag_t[nli][NGROUP - 1][ds(toff, npc_pad - npc), :],
                        ztile[0:npc_pad - npc, :])
                    nc.sync.dma_start(tab_t[nli][ntab:ntab + 1, :],
                                      zrow[:, 0:128])
                chunk_ptr = [0]   # next hag chunk to emit
                group_ptr = [0]   # next collective group to emit
                done_pos = [0]    # aggregated node-position high-water mark
                slack = [0]       # defer emission by one super-group

                def drain_table(limit_pos):
                    while (chunk_ptr[0] < nchunks
                           and (chunk_ptr[0] * 128 + 128 <= limit_pos
                                or limit_pos >= npc)):
                        emit_table_chunk(nli, chunk_ptr[0])
                        chunk_ptr[0] += 1
                    while (group_ptr[0] < NGROUP
                           and chunk_ptr[0] * 128 >= min(
                               (group_ptr[0] + 1) * GSZ, npc)):
                        emit_group_collective(nli, group_ptr[0])
                        group_ptr[0] += 1

                # ---- edge phase ----
                ba = ba_s[li]
                hprev = (xT_s if li == 0 else
                         hstack[0:64, 0, :] if li == 1 else hstack[:, 1, :])
                wd = wd_s[li]
                for st0 in (range(0, ntiles, SUPER) if with_edge else []):
                    g = min(SUPER, ntiles - st0)
                    nidx = g * TSLOT
                    vg = gat.tile([128, 1, nidx], BF16, tag="vg")
                    c0 = st0 * TSLOT // 16
                    nc.gpsimd.dma_gather(
                        vg, tab[...],
                        vidx_s[:, ds(c0, nidx // 16)],
                        nidx, nidx, elem_size=128, transpose=True,
                        single_packet=False, queue_num=(gq[0] % NQ) if NQ > 1 else 0)
                    gq[0] += 1
                    if produce:
                        # emit table work for positions aggregated one
                        # super-group ago (keeps collective waits short)
                        drain_table(slack[0])
                        slack[0] = done_pos[0]
                    for tt in range(g):
                        d, n_t, pos0 = plan.tiles[st0 + tt]
                        T = n_t * d
                        o = tt * TSLOT
                        # pre-activation: psum = Wd^T h_dst(bcast) + Wb^T h_src
                        pp = psp.tile([128, J, TSLOT], F32, tag="pre_ps2")
                        hb = hprev[:, ds(pos0, n_t)].rearrange(
                            "p (nt one) -> p nt one", one=1).broadcast_to(
                                (hprev.shape[0], n_t, d))
                        for jj in range(J):
                            nc.tensor.matmul(
                                pp[:, jj, 0:T], wd[:, ds(jj * 128, 128)], hb,
                                start=True, stop=False)
                            nc.tensor.matmul(
                                pp[:, jj, 0:T], wb_s[li][:, ds(jj * 128, 128)],
                                vg[:, 0, ds(o, T)],
                                start=False, stop=True)
                        # relu + first bias, evict to bf16
                        rl = edg.tile([128, J, TSLOT], BF16, tag="rl")
                        for jj in range(J):
                            nc.scalar.activation(
                                rl[:, jj, 0:T], pp[:, jj, 0:T],
                                mybir.ActivationFunctionType.Relu,
                                bias=ba[:, jj:jj + 1])
                        ps = psp.tile([128, J, TSLOT], F32, tag="edge_ps")
                        for jj in range(J):
                            for kk in range(J):
                                w = (we_s[li][...] if J == 1
                                     else we_s[li][:, kk, ds(jj * 128, 128)])
                                nc.tensor.matmul(
                                    ps[:, jj, 0:T], w, rl[:, kk, 0:T],
                                    start=(kk == 0), stop=(kk == J - 1))
                        tmp = edg.tile([128, J, 128], BF16, tag="agg")
                        for jj in range(J):
                            nc.vector.reduce_max(
                                tmp[:, jj, 0:n_t],
                                ps[:, jj, 0:T].rearrange(
                                    "p (nt d) -> p nt d", d=d),
                                axis=mybir.AxisListType.X)
                        for jj in range(J):
                            nc.vector.tensor_add(
                                hstack[:, L["out_chunks"][jj], ds(pos0, n_t)],
                                tmp[:, jj, 0:n_t],
                                bm_s[li][:, jj, ds(pos0, n_t)])
                        done_pos[0] = pos0 + n_t
                if produce:
                    drain_table(npc)   # finish remaining chunks + groups

            # =============== head ===============
            _ps_stack.close()
            with tc.tile_pool(name="psum1", bufs=2, space="PSUM") as psp1:
                outsb = sing.tile([4, npc], F32)
                if not with_head:
                    nc.vector.tensor_copy(outsb, hstack[0:4, 0, :])
                    nc.sync.dma_start(outT[...], outsb)
                HW = 256
                for c0 in (range(0, npc, HW) if with_head else []):
                    w = min(HW, npc - c0)
                    ps1 = psp1.tile([128, 4, HW], F32, tag="h_ps1")
                    for jj in range(4):
                        for kk in range(4):
                            nc.tensor.matmul(
                                ps1[:, jj, 0:w], wh_s[0][:, kk, ds(jj * 128, 128)],
                                hstack[:, kk, ds(c0, w)],
                                start=(kk == 0), stop=(kk == 3))
                    m1 = edg.tile([128, 4, HW], BF16, tag="h_m1")
                    for jj in range(4):
                        nc.scalar.activation(
                            m1[:, jj, 0:w], ps1[:, jj, 0:w],
                            mybir.ActivationFunctionType.Relu,
                            bias=bh_s[0][:, jj:jj + 1])
                    ps2 = psp1.tile([128, 2, HW], F32, tag="h_ps2")
                    for jj in range(2):
                        for kk in range(4):
                            nc.tensor.matmul(
                                ps2[:, jj, 0:w], wh_s[1][:, kk, ds(jj * 128, 128)],
                                m1[:, kk, 0:w],
                                start=(kk == 0), stop=(kk == 3))
                    m2 = edg.tile([128, 2, HW], BF16, tag="h_m2")
                    for jj in range(2):
                        nc.scalar.activation(
                            m2[:, jj, 0:w], ps2[:, jj, 0:w],
                            mybir.ActivationFunctionType.Relu,
                            bias=bh_s[1][:, jj:jj + 1])
                    ps3 = psp1.tile([4, HW], F32, tag="h_ps3")
                    for kk in range(2):
                        nc.tensor.matmul(ps3[:, 0:w], wh_s[2][:, kk, :],
                                         m2[:, kk, 0:w],
                                         start=(kk == 0), stop=(kk == 1))
                    nc.scalar.activation(
                        outsb[:, ds(c0, w)], ps3[:, 0:w],
                        mybir.ActivationFunctionType.Identity,
                        bias=bh_s[2][:, 0:1])
                if with_head:
                    nc.sync.dma_start(outT[...], outsb)

    nc.compile()
    return nc


# ----------------------------------------------------------------------------
# entry point
# ----------------------------------------------------------------------------

class Runner:
    """Compile once; keep the jitted sharded executable for cheap re-runs."""

    def __init__(self, nc):
        import jax
        from jax.sharding import Mesh, PartitionSpec
        from jax.experimental.shard_map import shard_map

        bass2jax.install_neuronx_cc_hook()
        self.nc = nc
        partition_name = (nc.partition_id_tensor.name
                          if nc.partition_id_tensor else None)
        in_names, out_names, out_avals, zero_outs = [], [], [], []
        for alloc in nc.m.functions[0].allocations:
            if not isinstance(alloc, mybir.MemoryLocationSet):
                continue
            name = alloc.memorylocations[0].name
            if alloc.kind == "ExternalInput":
                if name != partition_name:
                    in_names.append(name)
            elif alloc.kind == "ExternalOutput":
                shape = tuple(alloc.tensor_shape)
                dtype = mybir.dt.np(alloc.dtype)
                out_names.append(name)
                out_avals.append(jax.core.ShapedArray(shape, dtype))
                zero_outs.append(np.zeros(shape, dtype))
        n_params = len(in_names)
        all_in = in_names + out_names
        if partition_name is not None:
            all_in.append(partition_name)
        self.in_names = in_names
        self.out_names = out_names
        self.out_avals = out_avals
        self.zero_outs = zero_outs

        def _body(*args):
            operands = list(args)
            if partition_name is not None:
                operands.append(bass2jax.partition_id_tensor())
            return tuple(bass2jax._bass_exec_p.bind(
                *operands, out_avals=tuple(out_avals),
                in_names=tuple(all_in), out_names=tuple(out_names),
                lowering_input_output_aliases=(),
                sim_require_finite=True, sim_require_nnan=True, nc=nc))

        devices = jax.devices()[:NCORES]
        mesh = Mesh(np.asarray(devices), ("core",))
        self.mesh = mesh
        n_outs = len(out_names)
        self.sharded = jax.jit(
            shard_map(_body, mesh=mesh,
                      in_specs=(PartitionSpec("core"),) * (n_params + n_outs),
                      out_specs=(PartitionSpec("core"),) * n_outs,
                      check_rep=False),
            keep_unused=True)

    def prepare(self, in_maps):
        """Upload per-core inputs to the devices once; returns dev args."""
        import jax
        from jax.sharding import NamedSharding, PartitionSpec
        sh = NamedSharding(self.mesh, PartitionSpec("core"))
        concat_in = [
            np.concatenate([np.asarray(in_maps[c][nm])
                            for c in range(NCORES)], axis=0)
            for nm in self.in_names]
        concat_zeros = [np.zeros((NCORES * z.shape[0], *z.shape[1:]), z.dtype)
                        for z in self.zero_outs]
        args = [jax.device_put(a, sh) for a in concat_in + concat_zeros]
        jax.block_until_ready(args)
        return args

    def run(self, dev_args):
        import jax
        outs = self.sharded(*dev_args)
        jax.block_until_ready(outs)
        return outs

    def __call__(self, in_maps):
        outs = self.run(self.prepare(in_maps))
        return [
            {nm: np.asarray(outs[i]).reshape(
                NCORES, *self.out_avals[i].shape)[c]
             for i, nm in enumerate(self.out_names)}
            for c in range(NCORES)]


_CACHE = {}


def get_compiled(inputs: dict):
    x = np.asarray(inputs["x"])
    ei = np.asarray(inputs["edge_index"])
    n = x.shape[0]
    key = (n, ei.shape[1], hash(ei.tobytes()))
    if key not in _CACHE:
        plan = make_plan(n, ei)
        nc = build_program(plan)
        _CACHE.clear()
        _CACHE[key] = (plan, Runner(nc))
    return _CACHE[key]


def _run(inputs: dict) -> np.ndarray:
    plan, runner = get_compiled(inputs)
    in_maps = prep_inputs(inputs, plan)
    results = runner(in_maps)
    npc, n = plan.npc, plan.n
    out = np.empty((n, 4), np.float32)
    for c in range(NCORES):
        out[plan.perm[c * npc:(c + 1) * npc]] = results[c]["outT"].T
    return out


def kernel(**inputs) -> np.ndarray:
    return _run(inputs)


# revision 15
# speedup vs baseline: 102.2671x; 2.0318x over previous
"""DGCNN segmentation (3x EdgeConv max-aggregation + MLP head) on 8 Trainium2 cores.

Sharding: nodes are split into 8 equal contiguous blocks (one per core); each
core owns all edges whose *destination* lies in its block, so the scatter-max
aggregation is core-local.  Per-layer node tables are AllGather'd in 5
row-groups (overlapped with the producing edge phase) so every core can gather
any source node's row.

Per layer l (C_in -> C -> C, PyG EdgeConv):
    m_e   = relu(u[dst_e] + v[src_e] + ba_l) @ Wb_l          (per edge)
    h_i   = max_{e: dst_e = i} m_e + bb_l   (0 if no edges)
  where u = h @ (Wa_l[:C_in] - Wa_l[C_in:]),  v = h @ Wa_l[C_in:].

Device pipeline per core:
  - node row-tables in HBM (bf16, grouped layout); per-edge transposed gather
    via gpsimd.dma_gather(transpose=True) puts channels on partitions.
    Gathers round-robin over 4 SWDGE queues (4 descriptor rings drain in
    parallel: ~4x the single-ring gather bandwidth).
  - PE matmuls compute Wd^T h_dst (broadcast over each node's slot run) +
    Wb^T h_src into PSUM, scalar-engine relu+bias, PE matmul with We, then a
    segmented max over each node's padded slot-block on the vector engine.
  - Edges are pre-sorted by destination and padded so each node owns a
    fixed-width slot run inside a 512-slot tile (identical tile structure on
    all 8 cores; only index data differs - the program is pure SPMD).
"""

import os
from dataclasses import dataclass, field

import numpy as np

import concourse.bass as bass
import concourse.mybir as mybir
import concourse.bacc as bacc
import concourse.tile as tile
from concourse import bass_utils, bass2jax
from concourse.bass import ds

F32 = mybir.dt.float32
BF16 = mybir.dt.bfloat16
I16 = mybir.dt.int16

NCORES = 8
TSLOT = 512          # edge-slots per tile (== max matmul moving free dim)
SUPER = 4            # tiles per dma_gather call
NQ = 4               # SWDGE queues used round-robin for gathers
NGROUP = 5           # table AllGather row-groups
GSZ = 768            # rows per group (npc padded to NGROUP*GSZ = 3840)


# ----------------------------------------------------------------------------
# host-side preprocessing
# ----------------------------------------------------------------------------

@dataclass
class Plan:
    n: int
    npc: int
    npc_pad: int          # NGROUP * GSZ
    ntab: int             # NCORES * npc_pad (+1 sentinel row at index ntab)
    tiles: list  # list of (D, n_t, pos0)  shared by all cores
    S: int       # total slots = TSLOT * len(tiles)
    perm: np.ndarray      # new position -> old node id
    vidx: list = field(default_factory=list)   # per-core wrapped [128, S/16] i16
    has_iso: bool = False  # any zero-degree node anywhere


def _remap_rows(pos, npc, npc_pad):
    """Map node position (c*npc + r) -> grouped table row id."""
    c, r = pos // npc, pos % npc
    g = r // GSZ
    return g * (NCORES * GSZ) + c * GSZ + (r - g * GSZ)


def make_plan(n: int, edge_index: np.ndarray) -> Plan:
    assert n % NCORES == 0
    npc = n // NCORES
    npc_pad = NGROUP * GSZ
    assert npc <= npc_pad
    ntab = NCORES * npc_pad
    src = np.asarray(edge_index[0], dtype=np.int64)
    dst = np.asarray(edge_index[1], dtype=np.int64)
    deg = np.bincount(dst, minlength=n)

    # per-core block, degree-sorted (desc) within block
    perm = np.concatenate(
        [c * npc + np.argsort(-deg[c * npc:(c + 1) * npc], kind="stable")
         for c in range(NCORES)]
    )
    inv = np.empty(n, np.int64)
    inv[perm] = np.arange(n)
    src_n = inv[src]
    dst_n = inv[dst]
    deg_n = deg[perm]

    # shared tile structure from the max degree profile across cores
    degm = deg_n.reshape(NCORES, npc)
    maxdeg = degm.max(axis=0)
    tiles = []
    pos = 0
    while pos < npc:
        d = int(maxdeg[pos])
        d = max(2, d + (d & 1))          # even, >= 2
        n_t = min(TSLOT // d, npc - pos)
        tiles.append((d, n_t, pos))
        pos += n_t
    S = TSLOT * len(tiles)

    plan = Plan(n=n, npc=npc, npc_pad=npc_pad, ntab=ntab, tiles=tiles, S=S,
                perm=perm)
    plan.has_iso = bool((deg == 0).any())

    # grouped-table row id for each node position
    rowid = _remap_rows(np.arange(n), npc, npc_pad)

    # per-core slot fill
    order = np.argsort(dst_n, kind="stable")
    src_s = src_n[order]
    dst_s = dst_n[order]
    starts = np.searchsorted(dst_s, np.arange(n))       # per new-id start
    for c in range(NCORES):
        dloc = deg_n[c * npc:(c + 1) * npc]
        vfill = np.full(npc, ntab, np.int64)   # sentinel: zero row
        nz = dloc > 0
        gids = c * npc + np.arange(npc)
        vfill[nz] = rowid[src_s[starts[gids[nz]]]]  # first in-edge's src row

        vidx = np.full(S, ntab, np.int64)
        base_pos = np.empty(npc, np.int64)
        for ti, (d, n_t, pos0) in enumerate(tiles):
            sl0 = ti * TSLOT
            p = np.arange(pos0, pos0 + n_t)
            base_pos[p] = sl0 + (p - pos0) * d
            vidx[sl0:sl0 + n_t * d] = np.repeat(vfill[p], d)
        # overwrite real edges
        m = (dst_s >= c * npc) & (dst_s < (c + 1) * npc)
        es, ed = src_s[m], dst_s[m] - c * npc
        # rank within node: edges of a node are contiguous since sorted by dst
        rank = np.arange(len(ed)) - np.searchsorted(ed, ed)
        slots = base_pos[ed] + rank
        vidx[slots] = rowid[es]

        def wrap(a):
            w = a.astype(np.int16).reshape(-1, 16).T   # [16, S/16]
            return np.tile(w, (8, 1)).copy()           # [128, S/16]
        plan.vidx.append(wrap(vidx))
    return plan


def prep_inputs(inputs: dict, plan: Plan) -> list:
    """Build per-core in_maps (keys = dram tensor names)."""
    n, npc, perm = plan.n, plan.npc, plan.perm
    f32 = np.float32
    import ml_dtypes
    bf16 = ml_dtypes.bfloat16

    x = np.asarray(inputs["x"], f32)[perm]              # [n, 3] permuted
    deg = np.bincount(np.asarray(inputs["edge_index"][1]), minlength=n)
    mask = (deg[perm] > 0).astype(f32)                  # new order

    def lin(pref):
        wa = np.asarray(inputs[f"w{pref}a"], f32)
        ba = np.asarray(inputs[f"b{pref}a"], f32)
        wb = np.asarray(inputs[f"w{pref}b"], f32)
        bb = np.asarray(inputs[f"b{pref}b"], f32)
        return wa, ba, wb, bb

    w1a, b1a, w1b, b1b = lin("1")
    w2a, b2a, w2b, b2b = lin("2")
    w3a, b3a, w3b, b3b = lin("3")
    wm1 = np.asarray(inputs["wm1"], f32); bm1 = np.asarray(inputs["bm1"], f32)
    wm2 = np.asarray(inputs["wm2"], f32); bm2 = np.asarray(inputs["bm2"], f32)
    wm3 = np.asarray(inputs["wm3"], f32); bm3 = np.asarray(inputs["bm3"], f32)

    # per-layer split weights  Wd = Wa[:cin]-Wa[cin:],  Wb = Wa[cin:]
    # edge tables hold raw h rows (128-padded); both halves of the first
    # linear run on the PE per edge-tile.
    def wsplit(wa, cin, cmid, cp):
        wd = np.zeros((cin, cp), f32); wb = np.zeros((128, cp), f32)
        wd[:, :cmid] = wa[:cin] - wa[cin:]
        wb[:cin, :cmid] = wa[cin:]
        return wd, wb
    wd1, wb1 = wsplit(w1a, 3, 64, 128)
    wd2, wb2 = wsplit(w2a, 64, 128, 128)
    wd3, wb3 = wsplit(w3a, 128, 256, 256)

    # edge matmul weights (second linear), padded, bf16
    we1 = np.zeros((128, 128), f32); we1[0:64, 0:64] = w1b
    we2 = w2b.astype(f32)
    we3 = w3b.reshape(2, 128, 256).astype(f32)          # [k, 128, 256]

    ba1 = np.zeros((128, 1), f32); ba1[0:64, 0] = b1a
    ba2 = b2a.reshape(128, 1).astype(f32)
    ba3 = b3a.reshape(2, 128).T.astype(f32)             # [128, 2]

    # L1 gather table: x rows padded to 128 cols, grouped row layout
    xtab = np.zeros((plan.ntab + 1, 128), f32)
    rowid = _remap_rows(np.arange(n), npc, plan.npc_pad)
    xtab[rowid, 0:3] = x
    xtab = xtab.astype(bf16)

    # bmask_l [128, J, npc] = bb[c] * mask[n]
    def bmask(bb, cmid, j, mloc):
        bpad = np.zeros(128 * j, f32)
        bpad[:cmid] = bb
        out = bpad.reshape(j, 128).transpose(1, 0)[:, :, None] * mloc[None, None, :]
        return np.ascontiguousarray(out, dtype=bf16)

    # head weights: rearrange wm1 rows to hstack layout [h1(64) 0(64) h2 h3]
    wm1_arr = np.zeros((512, 512), f32)
    wm1_arr[0:64] = wm1[0:64]
    wm1_arr[128:256] = wm1[64:192]
    wm1_arr[256:512] = wm1[192:448]

    in_maps = []
    for c in range(NCORES):
        mloc = mask[c * npc:(c + 1) * npc]
        m = {
            "xT": np.ascontiguousarray(
                x[c * npc:(c + 1) * npc].T).astype(bf16),   # [3, npc]
            "xtab": xtab,
            "vidx": plan.vidx[c],
            "wd1": wd1.astype(bf16), "wd2": wd2.astype(bf16),
            "wd3": wd3.astype(bf16),
            "wb1": wb1.astype(bf16), "wb2": wb2.astype(bf16),
            "wb3": wb3.astype(bf16),
            "we1": we1.astype(bf16), "we2": we2.astype(bf16),
            "we3": np.ascontiguousarray(we3.transpose(1, 0, 2)).astype(bf16),
            "ident": np.eye(128, dtype=np.float32).astype(bf16),
            "ba1": ba1, "ba2": ba2, "ba3": ba3,
            "bm1": bmask(b1b, 64, 1, mloc),
            "bm2": bmask(b2b, 128, 1, mloc),
            "bm3": bmask(b3b, 256, 2, mloc),
            "wh1": np.ascontiguousarray(
                wm1_arr.reshape(4, 128, 512).transpose(1, 0, 2)).astype(bf16),
            "wh2": np.ascontiguousarray(
                wm2.reshape(4, 128, 256).transpose(1, 0, 2)).astype(bf16),
            "wh3": np.ascontiguousarray(
                wm3.reshape(2, 128, 4).transpose(1, 0, 2)).astype(bf16),
            "bh1": np.ascontiguousarray(bm1.reshape(4, 128).T),
            "bh2": np.ascontiguousarray(bm2.reshape(2, 128).T),
            "bh3": bm3.reshape(4, 1).astype(f32),
        }
        in_maps.append(m)
    return in_maps


# ----------------------------------------------------------------------------
# device program
# ----------------------------------------------------------------------------

LAYERS = [
    # (name, C_in, C_mid(padded J*128), J, hs_lhs(prev h chunk), hs_out(j->chunk))
    dict(name="1", cin=3, cp=128, j=1, out_chunks=[0]),
    dict(name="2", cin=64, cp=128, j=1, out_chunks=[1]),
    dict(name="3", cin=128, cp=256, j=2, out_chunks=[2, 3]),
]


def build_program(plan: Plan, nlayers: int = 3, with_head: bool = True,
                  with_edge: bool = True, timeline: bool = False):
    n, npc, S = plan.n, plan.npc, plan.S
    npc_pad, ntab = plan.npc_pad, plan.ntab
    ntiles = len(plan.tiles)
    nc = bacc.Bacc(
        "TRN2", target_bir_lowering=False, debug=False,
        enable_asserts=False, num_devices=1 if timeline else NCORES,
        num_swdge_queues=NQ,
    )
    RG = [list(range(NCORES))]

    # ---- dram tensors -------------------------------------------------------
    din = {}
    def dram_in(name, shape, dt):
        din[name] = nc.dram_tensor(name, list(shape), dt, kind="ExternalInput")
        return din[name]

    xT = dram_in("xT", (3, npc), BF16)
    xtab_d = dram_in("xtab", (ntab + 1, 128), BF16)
    vidx_d = dram_in("vidx", (128, S // 16), I16)
    wd_d = [dram_in("wd1", (3, 128), BF16), dram_in("wd2", (64, 128), BF16),
            dram_in("wd3", (128, 256), BF16)]
    wb_d = [dram_in("wb1", (128, 128), BF16), dram_in("wb2", (128, 128), BF16),
            dram_in("wb3", (128, 256), BF16)]
    we_d = [dram_in("we1", (128, 128), BF16), dram_in("we2", (128, 128), BF16),
            dram_in("we3", (128, 2, 256), BF16)]
    ident_d = dram_in("ident", (128, 128), BF16)
    ba_d = [dram_in("ba1", (128, 1), F32), dram_in("ba2", (128, 1), F32),
            dram_in("ba3", (128, 2), F32)]
    bm_d = [dram_in("bm1", (128, 1, npc), BF16),
            dram_in("bm2", (128, 1, npc), BF16),
            dram_in("bm3", (128, 2, npc), BF16)]
    wh_d = [dram_in("wh1", (128, 4, 512), BF16),
            dram_in("wh2", (128, 4, 256), BF16),
            dram_in("wh3", (128, 2, 4), BF16)]
    bh_d = [dram_in("bh1", (128, 4), F32), dram_in("bh2", (128, 2), F32),
            dram_in("bh3", (4, 1), F32)]
    outT = nc.dram_tensor("outT", [4, npc], F32, kind="ExternalOutput")

    # internal row tables of h_(l-1) for layers 2,3 (l=1 uses xtab input).
    # NOTE: dma_gather cannot read Shared-addr-space scratchpad; keep Local.
    # One hag tensor per AllGather row-group: each collective then reads a
    # full tensor (whole-tensor dep tracking; sliced collective ins raced).
    hag_t = [None] + [[nc.dram_tensor(f"hag{i}g{g}", [GSZ, 128], BF16,
                                      kind="Internal") for g in range(NGROUP)]
                      for i in (2, 3)]
    tab_t = [None] + [nc.dram_tensor(f"tab{i}", [ntab + 1, 128], BF16,
                                     kind="Internal") for i in (2, 3)]

    with tile.TileContext(nc) as tc:
        with (
            tc.tile_pool(name="singles", bufs=1) as sing,
            tc.tile_pool(name="stage", bufs=3) as stg,
            tc.tile_pool(name="gather", bufs=6) as gat,
            tc.tile_pool(name="edge", bufs=3) as edg,
        ):
            # ---- load constants into SBUF ----
            def load(dt_handle, shape, dtype, tag):
                t = sing.tile(list(shape), dtype, tag=tag)
                nc.sync.dma_start(t, dt_handle[...])
                return t

            xT_s = load(xT, (3, npc), BF16, "xT")
            vidx_s = load(vidx_d, (128, S // 16), I16, "vidx")
            wd_s = [load(wd_d[0], (3, 128), BF16, "wd0"),
                    load(wd_d[1], (64, 128), BF16, "wd1"),
                    load(wd_d[2], (128, 256), BF16, "wd2")]
            wb_s = [load(wb_d[i], wb_d[i].shape, BF16, f"wb{i}")
                    for i in range(3)]
            ident = load(ident_d, (128, 128), BF16, "ident")
            we_s = [load(we_d[i], we_d[i].shape, BF16, f"we{i}")
                    for i in range(3)]
            ba_s = [load(ba_d[i], ba_d[i].shape, F32, f"ba{i}")
                    for i in range(3)]
            bm_s = [load(bm_d[i], bm_d[i].shape, BF16, f"bm{i}")
                    for i in range(3)]
            wh_s = [load(wh_d[i], wh_d[i].shape, BF16, f"wh{i}")
                    for i in range(3)]
            bh_s = [load(bh_d[i], bh_d[i].shape, F32, f"bh{i}")
                    for i in range(3)]

            hstack = sing.tile([128, 4, npc], BF16)     # [h1;0 | h2 | h3a | h3b]
            if not with_edge:
                nc.vector.memset(hstack, 0.0)
            zrow = sing.tile([1, 512], BF16, tag="zrow")
            nc.vector.memset(zrow, 0.0)
            ztile = sing.tile([128, 128], BF16, tag="ztile")
            nc.vector.memset(ztile, 0.0)

            # =============== per layer ===============
            from contextlib import ExitStack
            _ps_stack = ExitStack()
            psp = _ps_stack.enter_context(
                tc.tile_pool(name="psum", bufs=3, space="PSUM"))
            psT = _ps_stack.enter_context(
                tc.tile_pool(name="psumT", bufs=2, space="PSUM"))
            gq = [0]  # round-robin gather queue counter
            nchunks = (npc + 127) // 128

            # table production for layer li+1, interleaved into layer li's
            # edge loop: transpose 128-node chunks of this layer's output as
            # they aggregate, DMA to hag, and fire each row-group's AllGather
            # as soon as its chunks have landed.
            def emit_table_chunk(nli, cn):
                """Transpose+store hag chunk cn of the table for layer nli."""
                r0 = cn * 128
                m = min(128, npc - r0)
                grp = r0 // GSZ            # GSZ % 128 == 0: chunk in 1 group
                go = r0 - grp * GSZ
                if nli == 1:
                    lhsT = hstack[0:64, 0, ds(r0, m)]
                    kdim = 64
                else:
                    lhsT = hstack[:, 1, ds(r0, m)]
                    kdim = 128
                ps = psp.tile([128, 128], F32, tag="edge_ps")
                nc.tensor.matmul(ps[:m], lhsT, ident[:kdim, :],
                                 start=True, stop=True)
                st = stg.tile([128, 128], BF16, tag="uv_stage")
                nc.scalar.copy(st[:m], ps[:m])
                nc.sync.dma_start(hag_t[nli][grp][ds(go, m), :], st[:m])

            def emit_group_collective(nli, g):
                o0 = g * (NCORES * GSZ)
                if timeline:
                    nc.sync.dma_start(
                        tab_t[nli][ds(o0, GSZ), :], hag_t[nli][g][...])
                else:
                    nc.gpsimd.collective_compute(
                        "AllGather", mybir.AluOpType.bypass, RG,
                        ins=[hag_t[nli][g][...]],
                        outs=[tab_t[nli][ds(o0, NCORES * GSZ), :]],
                    )

            for li, L in enumerate(LAYERS[:nlayers]):
                cin, cp, J = L["cin"], L["cp"], L["j"]
                tab = xtab_d if li == 0 else tab_t[li]

                # next layer's table bookkeeping
                nli = li + 1
                produce = with_edge and nli < nlayers and nli > 0
                if produce:
                    # zero padded tail rows [npc, npc_pad) once, up front
                    # (rows 678..768 of the last group's hag tensor)
                    toff = npc - (NGROUP - 1) * GSZ
                    nc.sync.dma_start(
                        hag_t[nli][NGROUP - 1][ds(toff, npc_pad - npc), :],
                        ztile[0:npc_pad - npc, :])
                    nc.sync.dma_start(tab_t[nli][ntab:ntab + 1, :],
                                      zrow[:, 0:128])
                chunk_ptr = [0]   # next hag chunk to emit
                group_ptr = [0]   # next collective group to emit
                done_pos = [0]    # aggregated node-position high-water mark
                slack = [0]       # defer emission by one super-group

                def drain_table(limit_pos):
                    while (chunk_ptr[0] < nchunks
                           and (chunk_ptr[0] * 128 + 128 <= limit_pos
                                or limit_pos >= npc)):
                        emit_table_chunk(nli, chunk_ptr[0])
                        chunk_ptr[0] += 1
                    while (group_ptr[0] < NGROUP
                           and chunk_ptr[0] * 128 >= min(
                               (group_ptr[0] + 1) * GSZ, npc)):
                        emit_group_collective(nli, group_ptr[0])
                        group_ptr[0] += 1

                # ---- edge phase ----
                ba = ba_s[li]
                hprev = (xT_s if li == 0 else
                         hstack[0:64, 0, :] if li == 1 else hstack[:, 1, :])
                wd = wd_s[li]
                for st0 in (range(0, ntiles, SUPER) if with_edge else []):
                    g = min(SUPER, ntiles - st0)
                    nidx = g * TSLOT
                    # non-transposed gather (multi-queue-safe; the transposed
                    # flavor corrupts data when issued on >1 SWDGE queue) +
                    # PE transpose to put channels on partitions.
                    vg = gat.tile([128, nidx // 128, 128], BF16, tag="vg")
                    c0 = st0 * TSLOT // 16
                    nc.gpsimd.dma_gather(
                        vg, tab[...],
                        vidx_s[:, ds(c0, nidx // 16)],
                        nidx, nidx, elem_size=128, transpose=False,
                        single_packet=False,
                        queue_num=(gq[0] % NQ) if NQ > 1 else 0)
                    gq[0] += 1
                    vgt = gat.tile([128, nidx], BF16, tag="vgt")
                    for b in range(nidx // TSLOT):
                        trp = psT.tile([128, TSLOT], BF16, tag="tr_ps")
                        for j4 in range(TSLOT // 128):
                            blk = b * (TSLOT // 128) + j4
                            nc.tensor.transpose(
                                trp[:, ds(j4 * 128, 128)], vg[:, blk, :],
                                ident)
                        nc.scalar.copy(vgt[:, ds(b * TSLOT, TSLOT)], trp)
                    if produce:
                        # emit table work for positions aggregated one
                        # super-group ago (keeps collective waits short)
                        drain_table(slack[0])
                        slack[0] = done_pos[0]
                    for tt in range(g):
                        d, n_t, pos0 = plan.tiles[st0 + tt]
                        T = n_t * d
                        o = tt * TSLOT
                        # pre-activation: psum = Wd^T h_dst(bcast) + Wb^T h_src
                        hb = hprev[:, ds(pos0, n_t)].rearrange(
                            "p (nt one) -> p nt one", one=1).broadcast_to(
                                (hprev.shape[0], n_t, d))
                        rl = edg.tile([128, J, TSLOT], BF16, tag="rl")
                        for jj in range(J):
                            pp = psp.tile([128, TSLOT], F32, tag="pre_ps")
                            nc.tensor.matmul(
                                pp[:, 0:T], wd[:, ds(jj * 128, 128)], hb,
                                start=True, stop=False)
                            nc.tensor.matmul(
                                pp[:, 0:T], wb_s[li][:, ds(jj * 128, 128)],
                                vgt[:, ds(o, T)],
                                start=False, stop=True)
                            nc.scalar.activation(
                                rl[:, jj, 0:T], pp[:, 0:T],
                                mybir.ActivationFunctionType.Relu,
                                bias=ba[:, jj:jj + 1])
                        tmp = edg.tile([128, J, 128], BF16, tag="agg")
                        for jj in range(J):
                            ps = psp.tile([128, TSLOT], F32, tag="edge_ps")
                            for kk in range(J):
                                w = (we_s[li][...] if J == 1
                                     else we_s[li][:, kk, ds(jj * 128, 128)])
                                nc.tensor.matmul(
                                    ps[:, 0:T], w, rl[:, kk, 0:T],
                                    start=(kk == 0), stop=(kk == J - 1))
                            nc.vector.reduce_max(
                                tmp[:, jj, 0:n_t],
                                ps[:, 0:T].rearrange(
                                    "p (nt d) -> p nt d", d=d),
                                axis=mybir.AxisListType.X)
                        for jj in range(J):
                            nc.vector.tensor_add(
                                hstack[:, L["out_chunks"][jj], ds(pos0, n_t)],
                                tmp[:, jj, 0:n_t],
                                bm_s[li][:, jj, ds(pos0, n_t)])
                        done_pos[0] = pos0 + n_t
                if produce:
                    drain_table(npc)   # finish remaining chunks + groups

            # =============== head ===============
            _ps_stack.close()
            with tc.tile_pool(name="psum1", bufs=2, space="PSUM") as psp1:
                outsb = sing.tile([4, npc], F32)
                if not with_head:
                    nc.vector.tensor_copy(outsb, hstack[0:4, 0, :])
                    nc.sync.dma_start(outT[...], outsb)
                HW = 256
                for c0 in (range(0, npc, HW) if with_head else []):
                    w = min(HW, npc - c0)
                    ps1 = psp1.tile([128, 4, HW], F32, tag="h_ps1")
                    for jj in range(4):
                        for kk in range(4):
                            nc.tensor.matmul(
                                ps1[:, jj, 0:w], wh_s[0][:, kk, ds(jj * 128, 128)],
                                hstack[:, kk, ds(c0, w)],
                                start=(kk == 0), stop=(kk == 3))
                    m1 = edg.tile([128, 4, HW], BF16, tag="h_m1")
                    for jj in range(4):
                        nc.scalar.activation(
                            m1[:, jj, 0:w], ps1[:, jj, 0:w],
                            mybir.ActivationFunctionType.Relu,
                            bias=bh_s[0][:, jj:jj + 1])
                    ps2 = psp1.tile([128, 2, HW], F32, tag="h_ps2")
                    for jj in range(2):
                        for kk in range(4):
                            nc.tensor.matmul(
                                ps2[:, jj, 0:w], wh_s[1][:, kk, ds(jj * 128, 128)],
                                m1[:, kk, 0:w],
                                start=(kk == 0), stop=(kk == 3))
                    m2 = edg.tile([128, 2, HW], BF16, tag="h_m2")
                    for jj in range(2):
                        nc.scalar.activation(
                            m2[:, jj, 0:w], ps2[:, jj, 0:w],
                            mybir.ActivationFunctionType.Relu,
                            bias=bh_s[1][:, jj:jj + 1])
                    ps3 = psp1.tile([4, HW], F32, tag="h_ps3")
                    for kk in range(2):
                        nc.tensor.matmul(ps3[:, 0:w], wh_s[2][:, kk, :],
                                         m2[:, kk, 0:w],
                                         start=(kk == 0), stop=(kk == 1))
                    nc.scalar.activation(
                        outsb[:, ds(c0, w)], ps3[:, 0:w],
                        mybir.ActivationFunctionType.Identity,
                        bias=bh_s[2][:, 0:1])
                if with_head:
                    nc.sync.dma_start(outT[...], outsb)

    nc.compile()
    return nc


# ----------------------------------------------------------------------------
# entry point
# ----------------------------------------------------------------------------

class Runner:
    """Compile once; keep the jitted sharded executable for cheap re-runs."""

    def __init__(self, nc):
        import jax
        from jax.sharding import Mesh, PartitionSpec
        from jax.experimental.shard_map import shard_map

        bass2jax.install_neuronx_cc_hook()
        self.nc = nc
        partition_name = (nc.partition_id_tensor.name
                          if nc.partition_id_tensor else None)
        in_names, out_names, out_avals, zero_outs = [], [], [], []
        for alloc in nc.m.functions[0].allocations:
            if not isinstance(alloc, mybir.MemoryLocationSet):
                continue
            name = alloc.memorylocations[0].name
            if alloc.kind == "ExternalInput":
                if name != partition_name:
                    in_names.append(name)
            elif alloc.kind == "ExternalOutput":
                shape = tuple(alloc.tensor_shape)
                dtype = mybir.dt.np(alloc.dtype)
                out_names.append(name)
                out_avals.append(jax.core.ShapedArray(shape, dtype))
                zero_outs.append(np.zeros(shape, dtype))
        n_params = len(in_names)
        all_in = in_names + out_names
        if partition_name is not None:
            all_in.append(partition_name)
        self.in_names = in_names
        self.out_names = out_names
        self.out_avals = out_avals
        self.zero_outs = zero_outs

        def _body(*args):
            operands = list(args)
            if partition_name is not None:
                operands.append(bass2jax.partition_id_tensor())
            return tuple(bass2jax._bass_exec_p.bind(
                *operands, out_avals=tuple(out_avals),
                in_names=tuple(all_in), out_names=tuple(out_names),
                lowering_input_output_aliases=(),
                sim_require_finite=True, sim_require_nnan=True, nc=nc))

        devices = jax.devices()[:NCORES]
        mesh = Mesh(np.asarray(devices), ("core",))
        self.mesh = mesh
        n_outs = len(out_names)
        self.sharded = jax.jit(
            shard_map(_body, mesh=mesh,
                      in_specs=(PartitionSpec("core"),) * (n_params + n_outs),
                      out_specs=(PartitionSpec("core"),) * n_outs,
                      check_rep=False),
            keep_unused=True)

    def prepare(self, in_maps):
        """Upload per-core inputs to the devices once; returns dev args."""
        import jax
        from jax.sharding import NamedSharding, PartitionSpec
        sh = NamedSharding(self.mesh, PartitionSpec("core"))
        concat_in = [
            np.concatenate([np.asarray(in_maps[c][nm])
                            for c in range(NCORES)], axis=0)
            for nm in self.in_names]
        concat_zeros = [np.zeros((NCORES * z.shape[0], *z.shape[1:]), z.dtype)
                        for z in self.zero_outs]
        args = [jax.device_put(a, sh) for a in concat_in + concat_zeros]
        jax.block_until_ready(args)
        return args

    def run(self, dev_args):
        import jax
        outs = self.sharded(*dev_args)
        jax.block_until_ready(outs)
        return outs

    def __call__(self, in_maps):
        outs = self.run(self.prepare(in_maps))
        return [
            {nm: np.asarray(outs[i]).reshape(
                NCORES, *self.out_avals[i].shape)[c]
             for i, nm in enumerate(self.out_names)}
            for c in range(NCORES)]


_CACHE = {}


def get_compiled(inputs: dict):
    x = np.asarray(inputs["x"])
    ei = np.asarray(inputs["edge_index"])
    n = x.shape[0]
    key = (n, ei.shape[1], hash(ei.tobytes()))
    if key not in _CACHE:
        plan = make_plan(n, ei)
        nc = build_program(plan)
        _CACHE.clear()
        _CACHE[key] = (plan, Runner(nc))
    return _CACHE[key]


def _run(inputs: dict) -> np.ndarray:
    plan, runner = get_compiled(inputs)
    in_maps = prep_inputs(inputs, plan)
    results = runner(in_maps)
    npc, n = plan.npc, plan.n
    out = np.empty((n, 4), np.float32)
    for c in range(NCORES):
        out[plan.perm[c * npc:(c + 1) * npc]] = results[c]["outT"].T
    return out


def kernel(**inputs) -> np.ndarray:
    return _run(inputs)
